# revision 13
# baseline (speedup 1.0000x reference)
"""Trainium2 Bass kernel for attention-weight computation.

Computes attn = softmax(encoder_outputs @ hidden) over seq_len=65536,
returning shape (1, 1, 65536) float32.

Distribution: encoder_outputs [65536, 1024] is sharded by rows across 8
NeuronCores (8192 rows each).  The host casts each slice to fp16 (accuracy
checked: rel_l2 ~3e-5 vs fp32 reference, far under the 2e-2 gate — softmax
normalization cancels the dominant entry's quantization error) and
pre-tiles it so every DMA reads fully contiguous 8 KiB partition lines:
per chunk c of `size` seq columns, the host stores [128, HC*size] fp16
where partition p holds h-chunk-major data etile[p, j*size+s] =
E[base+s, j*128+p].  The core streams these chunks from HBM and computes
its 8192 scores on the TensorEngine (hidden chunk = 1-column stationary
operand, chunk tile = [128, <=512] moving operand, accumulating the 8
h-chunks into a [1, size] PSUM tile).

Softmax stats are computed incrementally during the stream (per-chunk max
+ sum-of-exp directly from PSUM on the otherwise-idle Vector/Act
engines).  One AllGather of the first 17 chunks' (m, s) pairs overlaps
the stream tail; a second tiny AllGather covers the last (small) chunk.
The tail combines the 144 gathered pairs into the global max g and sum S,
then rescales the stored exp(s - m_t) rows by exp(m_t - g)/S split across
the Vector and Act engines, and writes the result out.
"""

import numpy as np

S_TOTAL = 65536
H = 1024
N_CORES = 8
S_PER = S_TOTAL // N_CORES  # 8192 rows per core
P = 128                     # SBUF partitions
HC = H // P                 # 8 h-chunks

# chunk sizes: small first chunks for pipeline ramp, small last chunk so
# the final stats (which gate the tail AllGather) are ready quickly.
# 512-col subchunks are processed in pairs sharing the h-loop (two PSUM
# accumulation chains) so consecutive matmuls hit different banks and
# pipeline instead of paying the isolated-matmul drain each time.
SIZES = [128, 256] + [512] * 14 + [256, 256, 128]
assert sum(SIZES) == S_PER
OFFS = [sum(SIZES[:i]) for i in range(len(SIZES))]
NCH = len(SIZES)            # 19 chunks
# DMA groups: chunk indices loaded in one dma_start (pairs of 512s)
DMA_GROUPS = [[0], [1], [2, 3], [4, 5], [6, 7], [8, 9], [10, 11], [12, 13],
              [14, 15], [16, 17], [18]]
assert sorted(c for g in DMA_GROUPS for c in g) == list(range(NCH))

_CACHE: dict = {}


def _build_module(mm_dtype: str = "float16"):
    import concourse.bacc as bacc
    import concourse.mybir as mybir
    import concourse.tile as tile

    fp32 = mybir.dt.float32
    mmdt = getattr(mybir.dt, mm_dtype)
    AX = mybir.AxisListType.X
    ALL_CORES = [list(range(N_CORES))]
    Act = mybir.ActivationFunctionType

    nc = bacc.Bacc(
        "TRN2",
        target_bir_lowering=False,
        debug=False,
        enable_asserts=False,
        num_devices=N_CORES,
    )

    # et: pre-tiled slice [P, HC*S_PER]; hc: hidden as [P, HC] (chunk j in col j)
    et = nc.dram_tensor("et", [P, HC * S_PER], mmdt, kind="ExternalInput").ap()
    hc = nc.dram_tensor("hc", [P, HC], mmdt, kind="ExternalInput").ap()
    out = nc.dram_tensor("out", [S_PER], fp32, kind="ExternalOutput").ap()

    with tile.TileContext(nc) as tc:
        with (
            tc.tile_pool(name="stream", bufs=6) as stream_pool,
            tc.tile_pool(name="persist", bufs=1) as persist_pool,
            tc.tile_pool(name="small", bufs=1) as small_pool,
            tc.tile_pool(name="psum", bufs=6, space="PSUM") as psum_pool,
            tc.tile_pool(name="dram", bufs=1, space="DRAM") as dram_pool,
        ):
            hid = small_pool.tile([P, HC], mmdt)
            nc.sync.dma_start(out=hid, in_=hc)

            exps_row = persist_pool.tile([1, S_PER], fp32)   # exp(s - m_t)
            attn_row = persist_pool.tile([1, S_PER], fp32)
            pair_row = small_pool.tile([1, 2 * NCH], fp32)   # (m_t, sum_t)
            negm_row = small_pool.tile([1, NCH], fp32)

            cc_in = dram_pool.tile([2 * NCH], fp32)
            cc_out = dram_pool.tile([N_CORES, 2 * NCH], fp32)

            # ---- stream chunks: matmul + incremental softmax stats ----
            for gi, group in enumerate(DMA_GROUPS):
                g0, gsz = OFFS[group[0]], sum(SIZES[c] for c in group)
                etile = stream_pool.tile(
                    [P, HC * gsz], mmdt, tag="et", bufs=6, name=f"et{gi}"
                )
                eng = nc.sync if gi % 2 == 0 else nc.scalar
                eng.dma_start(
                    out=etile, in_=et[:, HC * g0 : HC * (g0 + gsz)]
                )
                # interleave the chunks' PSUM chains so consecutive
                # matmuls target different banks and pipeline
                pss = {
                    c: psum_pool.tile([1, SIZES[c]], fp32, tag="ps", bufs=6,
                                      name=f"ps{c}")
                    for c in group
                }
                for j in range(HC):
                    for c in group:
                        base = HC * (OFFS[c] - g0) + j * SIZES[c]
                        nc.tensor.matmul(
                            pss[c],
                            hid[:, j : j + 1],
                            etile[:, base : base + SIZES[c]],
                            start=(j == 0),
                            stop=(j == HC - 1),
                        )
                for c in group:
                    # stats straight from PSUM; ACT writes exp row + sum
                    nc.vector.reduce_max(pair_row[:, 2 * c : 2 * c + 1], pss[c], axis=AX)
                    nc.vector.tensor_scalar_mul(
                        negm_row[:, c : c + 1], pair_row[:, 2 * c : 2 * c + 1], -1.0
                    )
                    nc.scalar.activation(
                        out=exps_row[:, OFFS[c] : OFFS[c] + SIZES[c]],
                        in_=pss[c],
                        func=Act.Exp,
                        bias=negm_row[:, c : c + 1],
                        scale=1.0,
                        accum_out=pair_row[:, 2 * c + 1 : 2 * c + 2],
                    )

            # ---- one AllGather of all (m, s) pairs; the sync HWDGE ring
            # is idle by now, so the bounce DMA completes fast ----
            nc.sync.dma_start(out=cc_in, in_=pair_row)
            nc.gpsimd.collective_compute(
                "AllGather",
                mybir.AluOpType.bypass,
                replica_groups=ALL_CORES,
                ins=[cc_in.opt()],
                outs=[cc_out.opt()],
            )

            # ---- speculative softmax from LOCAL stats, fully overlapped
            # with the AllGather flight; exact global fixup afterwards.
            # attn_true = exps*exp(m_t - b)/S_b for ANY reference b, so
            # rescale with local (b_l, S_l) now and multiply by
            # q = exp(b_l - b)*S_l/S_b once global stats arrive. ----
            pairv = pair_row.rearrange("o (k two) -> o two k", two=2)
            bl = small_pool.tile([1, 1], fp32)
            nc.vector.reduce_max(bl, pairv[:, 0, :], axis=AX)
            negbl = small_pool.tile([1, 1], fp32)
            nc.vector.tensor_scalar_mul(negbl, bl, -1.0)
            eml = small_pool.tile([1, NCH], fp32)
            nc.scalar.activation(
                out=eml, in_=pairv[:, 0, :], func=Act.Exp, bias=negbl, scale=1.0
            )
            termsl = small_pool.tile([1, NCH], fp32)
            sl = small_pool.tile([1, 1], fp32)
            nc.vector.tensor_mul(termsl, eml, pairv[:, 1, :])
            nc.vector.reduce_sum(sl, termsl, axis=AX)
            rsl = small_pool.tile([1, 1], fp32)
            nc.vector.reciprocal(rsl, sl)
            ftl = small_pool.tile([1, NCH], fp32)
            nc.vector.tensor_scalar_mul(ftl, eml, rsl)
            for c, size in enumerate(SIZES):
                src = exps_row[:, OFFS[c] : OFFS[c] + size]
                dst = attn_row[:, OFFS[c] : OFFS[c] + size]
                if c % 2 == 0:
                    nc.vector.tensor_scalar_mul(dst, src, ftl[:, c : c + 1])
                else:
                    nc.scalar.mul(dst, src, ftl[:, c : c + 1])

            # ---- global combine -> q, then one fixup multiply ----
            NP = N_CORES * 2 * NCH              # 304 gathered floats
            row = small_pool.tile([1, NP], fp32)
            nc.scalar.dma_start(out=row, in_=cc_out.rearrange("a b -> (a b)"))
            rowv = row.rearrange("o (k two) -> o two k", two=2)
            b1 = small_pool.tile([1, 1], fp32)
            nc.vector.reduce_max(b1, rowv[:, 0, :], axis=AX)
            negb1 = small_pool.tile([1, 1], fp32)
            nc.vector.tensor_scalar_mul(negb1, b1, -1.0)
            em = small_pool.tile([1, NP // 2], fp32)
            nc.scalar.activation(
                out=em, in_=rowv[:, 0, :], func=Act.Exp, bias=negb1, scale=1.0
            )
            terms = small_pool.tile([1, NP // 2], fp32)
            s1 = small_pool.tile([1, 1], fp32)
            nc.vector.tensor_mul(terms, em, rowv[:, 1, :])
            nc.vector.reduce_sum(s1, terms, axis=AX)
            rs1 = small_pool.tile([1, 1], fp32)
            nc.vector.reciprocal(rs1, s1)
            q0 = small_pool.tile([1, 1], fp32)
            nc.scalar.activation(
                out=q0, in_=bl, func=Act.Exp, bias=negb1, scale=1.0
            )
            q = small_pool.tile([1, 1], fp32)
            nc.vector.tensor_scalar(
                out=q, in0=q0, scalar1=sl, scalar2=rs1,
                op0=mybir.AluOpType.mult, op1=mybir.AluOpType.mult,
            )

            # fixup + output, quartered and split across engines/queues so
            # the out DMAs overlap the remaining fixup work
            fin_row = persist_pool.tile([1, S_PER], fp32)
            out2 = out.rearrange("(o s) -> o s", o=1)
            QN = S_PER // 4
            for i in range(4):
                src = attn_row[:, i * QN : (i + 1) * QN]
                dst = fin_row[:, i * QN : (i + 1) * QN]
                if i % 2 == 0:
                    nc.vector.tensor_scalar_mul(dst, src, q)
                else:
                    nc.scalar.mul(dst, src, q)
                deng = nc.sync if i % 2 == 0 else nc.scalar
                deng.dma_start(out=out2[:, i * QN : (i + 1) * QN], in_=dst)

    nc.compile()
    return nc


def _get_module():
    if "nc" not in _CACHE:
        _CACHE["nc"] = _build_module()
    return _CACHE["nc"]


def _prep_inputs(hidden: np.ndarray, encoder_outputs: np.ndarray):
    hidden = np.asarray(hidden, dtype=np.float32)
    eo = np.asarray(encoder_outputs, dtype=np.float32)
    h16 = hidden.astype(np.float16)
    eo16 = eo.astype(np.float16)
    hcm = np.ascontiguousarray(h16.reshape(HC, P).T)  # [P, HC]
    in_maps = []
    for c in range(N_CORES):
        es = eo16[c * S_PER : (c + 1) * S_PER]  # [S_PER, H]
        blocks = [
            es[OFFS[i] : OFFS[i] + SIZES[i], :]
            .reshape(SIZES[i], HC, P)
            .transpose(2, 1, 0)
            .reshape(P, HC * SIZES[i])
            for i in range(NCH)
        ]
        ets = np.ascontiguousarray(np.concatenate(blocks, axis=1))  # [P, HC*S_PER]
        in_maps.append({"et": ets, "hc": hcm})
    return in_maps


def _run(hidden: np.ndarray, encoder_outputs: np.ndarray, trace: bool = False):
    from concourse.bass_utils import run_bass_kernel_spmd

    nc = _get_module()
    in_maps = _prep_inputs(hidden, encoder_outputs)
    res = run_bass_kernel_spmd(
        nc, in_maps, core_ids=list(range(N_CORES)), trace=trace
    )
    parts = [np.asarray(res.results[c]["out"]).reshape(-1) for c in range(N_CORES)]
    attn = np.concatenate(parts)
    return attn.reshape(1, 1, S_TOTAL).astype(np.float32), res


def kernel(hidden: np.ndarray, encoder_outputs: np.ndarray) -> np.ndarray:
    try:
        out, _ = _run(hidden, encoder_outputs, trace=False)
    except Exception:
        # one retry for transient device/runtime hiccups
        _CACHE.clear()
        out, _ = _run(hidden, encoder_outputs, trace=False)
    return out


# revision 14
# speedup vs baseline: 1.1054x; 1.1054x over previous
"""Trainium2 Bass kernel for attention-weight computation.

Computes attn = softmax(encoder_outputs @ hidden) over seq_len=65536,
returning shape (1, 1, 65536) float32.

Distribution: encoder_outputs [65536, 1024] is sharded by rows across 8
NeuronCores (8192 rows each).  The host casts each slice to fp16 (accuracy
checked: rel_l2 ~3e-5 vs fp32 reference, far under the 2e-2 gate — softmax
normalization cancels the dominant entry's quantization error) and
pre-tiles it so every DMA reads fully contiguous 8 KiB partition lines:
per chunk c of `size` seq columns, the host stores [128, HC*size] fp16
where partition p holds h-chunk-major data etile[p, j*size+s] =
E[base+s, j*128+p].  The core streams these chunks from HBM and computes
its 8192 scores on the TensorEngine (hidden chunk = 1-column stationary
operand, chunk tile = [128, <=512] moving operand, accumulating the 8
h-chunks into a [1, size] PSUM tile).

Softmax stats are computed incrementally during the stream (per-chunk max
+ sum-of-exp directly from PSUM on the otherwise-idle Vector/Act
engines).  One AllGather of the first 17 chunks' (m, s) pairs overlaps
the stream tail; a second tiny AllGather covers the last (small) chunk.
The tail combines the 144 gathered pairs into the global max g and sum S,
then rescales the stored exp(s - m_t) rows by exp(m_t - g)/S split across
the Vector and Act engines, and writes the result out.
"""

import numpy as np

S_TOTAL = 65536
H = 1024
N_CORES = 8
S_PER = S_TOTAL // N_CORES  # 8192 rows per core
P = 128                     # SBUF partitions
HC = H // P                 # 8 h-chunks

# chunk sizes: small first chunks for pipeline ramp, small last chunk so
# the final stats (which gate the tail AllGather) are ready quickly.
# 512-col subchunks are processed in pairs sharing the h-loop (two PSUM
# accumulation chains) so consecutive matmuls hit different banks and
# pipeline instead of paying the isolated-matmul drain each time.
SIZES = [128, 256] + [512] * 14 + [256, 256, 128]
assert sum(SIZES) == S_PER
OFFS = [sum(SIZES[:i]) for i in range(len(SIZES))]
NCH = len(SIZES)            # 19 chunks
# DMA groups: chunk indices loaded in one dma_start (pairs of 512s)
DMA_GROUPS = [[0], [1], [2, 3], [4, 5], [6, 7], [8, 9], [10, 11], [12, 13],
              [14, 15], [16, 17], [18]]
assert sorted(c for g in DMA_GROUPS for c in g) == list(range(NCH))

_CACHE: dict = {}


def _build_module(mm_dtype: str = "float16"):
    import concourse.bacc as bacc
    import concourse.mybir as mybir
    import concourse.tile as tile

    fp32 = mybir.dt.float32
    mmdt = getattr(mybir.dt, mm_dtype)
    AX = mybir.AxisListType.X
    ALL_CORES = [list(range(N_CORES))]
    Act = mybir.ActivationFunctionType

    nc = bacc.Bacc(
        "TRN2",
        target_bir_lowering=False,
        debug=False,
        enable_asserts=False,
        num_devices=N_CORES,
    )

    # et: pre-tiled slice [P, HC*S_PER]; hc: hidden as [P, HC] (chunk j in col j)
    et = nc.dram_tensor("et", [P, HC * S_PER], mmdt, kind="ExternalInput").ap()
    hc = nc.dram_tensor("hc", [P, HC], mmdt, kind="ExternalInput").ap()
    out = nc.dram_tensor("out", [S_PER], fp32, kind="ExternalOutput").ap()

    with tile.TileContext(nc) as tc:
        with (
            tc.tile_pool(name="stream", bufs=6) as stream_pool,
            tc.tile_pool(name="persist", bufs=1) as persist_pool,
            tc.tile_pool(name="small", bufs=1) as small_pool,
            tc.tile_pool(name="psum", bufs=6, space="PSUM") as psum_pool,
            tc.tile_pool(name="dram", bufs=1, space="DRAM") as dram_pool,
        ):
            hid = small_pool.tile([P, HC], mmdt)
            nc.sync.dma_start(out=hid, in_=hc)

            exps_row = persist_pool.tile([1, S_PER], fp32)   # exp(s - m_t)
            attn_row = persist_pool.tile([1, S_PER], fp32)
            pair_row = small_pool.tile([1, 2 * NCH], fp32)   # (m_t, sum_t)
            negm_row = small_pool.tile([1, NCH], fp32)

            cc_in = dram_pool.tile([2 * NCH], fp32)
            cc_out = dram_pool.tile([N_CORES, 2 * NCH], fp32)

            # ---- stream chunks: matmul + incremental softmax stats ----
            for gi, group in enumerate(DMA_GROUPS):
                g0, gsz = OFFS[group[0]], sum(SIZES[c] for c in group)
                etile = stream_pool.tile(
                    [P, HC * gsz], mmdt, tag="et", bufs=6, name=f"et{gi}"
                )
                eng = nc.sync if gi % 2 == 0 else nc.scalar
                eng.dma_start(
                    out=etile, in_=et[:, HC * g0 : HC * (g0 + gsz)]
                )
                # interleave the chunks' PSUM chains so consecutive
                # matmuls target different banks and pipeline
                pss = {
                    c: psum_pool.tile([1, SIZES[c]], fp32, tag="ps", bufs=6,
                                      name=f"ps{c}")
                    for c in group
                }
                for j in range(HC):
                    for c in group:
                        base = HC * (OFFS[c] - g0) + j * SIZES[c]
                        nc.tensor.matmul(
                            pss[c],
                            hid[:, j : j + 1],
                            etile[:, base : base + SIZES[c]],
                            start=(j == 0),
                            stop=(j == HC - 1),
                        )
                for c in group:
                    # stats straight from PSUM; ACT writes exp row + sum
                    nc.vector.reduce_max(pair_row[:, 2 * c : 2 * c + 1], pss[c], axis=AX)
                    nc.vector.tensor_scalar_mul(
                        negm_row[:, c : c + 1], pair_row[:, 2 * c : 2 * c + 1], -1.0
                    )
                    nc.scalar.activation(
                        out=exps_row[:, OFFS[c] : OFFS[c] + SIZES[c]],
                        in_=pss[c],
                        func=Act.Exp,
                        bias=negm_row[:, c : c + 1],
                        scale=1.0,
                        accum_out=pair_row[:, 2 * c + 1 : 2 * c + 2],
                    )

            # ---- one AllGather of all (m, s) pairs; the sync HWDGE ring
            # is idle by now, so the bounce DMA completes fast ----
            nc.sync.dma_start(out=cc_in, in_=pair_row)
            nc.gpsimd.collective_compute(
                "AllGather",
                mybir.AluOpType.bypass,
                replica_groups=ALL_CORES,
                ins=[cc_in.opt()],
                outs=[cc_out.opt()],
            )

            # ---- speculative softmax from LOCAL stats, fully overlapped
            # with the AllGather flight; exact global fixup afterwards.
            # attn_true = exps*exp(m_t - b)/S_b for ANY reference b, so
            # rescale with local (b_l, S_l) now and multiply by
            # q = exp(b_l - b)*S_l/S_b once global stats arrive. ----
            pairv = pair_row.rearrange("o (k two) -> o two k", two=2)
            bl = small_pool.tile([1, 1], fp32)
            nc.vector.reduce_max(bl, pairv[:, 0, :], axis=AX)
            negbl = small_pool.tile([1, 1], fp32)
            nc.vector.tensor_scalar_mul(negbl, bl, -1.0)
            eml = small_pool.tile([1, NCH], fp32)
            nc.scalar.activation(
                out=eml, in_=pairv[:, 0, :], func=Act.Exp, bias=negbl, scale=1.0
            )
            termsl = small_pool.tile([1, NCH], fp32)
            sl = small_pool.tile([1, 1], fp32)
            nc.vector.tensor_mul(termsl, eml, pairv[:, 1, :])
            nc.vector.reduce_sum(sl, termsl, axis=AX)
            rsl = small_pool.tile([1, 1], fp32)
            nc.vector.reciprocal(rsl, sl)
            ftl = small_pool.tile([1, NCH], fp32)
            nc.vector.tensor_scalar_mul(ftl, eml, rsl)
            for c, size in enumerate(SIZES):
                src = exps_row[:, OFFS[c] : OFFS[c] + size]
                dst = attn_row[:, OFFS[c] : OFFS[c] + size]
                if c % 2 == 0:
                    nc.vector.tensor_scalar_mul(dst, src, ftl[:, c : c + 1])
                else:
                    nc.scalar.mul(dst, src, ftl[:, c : c + 1])

            # ---- global combine -> q, then one fixup multiply ----
            NP = N_CORES * 2 * NCH              # 304 gathered floats
            row = small_pool.tile([1, NP], fp32)
            nc.scalar.dma_start(out=row, in_=cc_out.rearrange("a b -> (a b)"))
            rowv = row.rearrange("o (k two) -> o two k", two=2)
            b1 = small_pool.tile([1, 1], fp32)
            nc.vector.reduce_max(b1, rowv[:, 0, :], axis=AX)
            negb1 = small_pool.tile([1, 1], fp32)
            nc.vector.tensor_scalar_mul(negb1, b1, -1.0)
            em = small_pool.tile([1, NP // 2], fp32)
            nc.scalar.activation(
                out=em, in_=rowv[:, 0, :], func=Act.Exp, bias=negb1, scale=1.0
            )
            terms = small_pool.tile([1, NP // 2], fp32)
            s1 = small_pool.tile([1, 1], fp32)
            nc.vector.tensor_mul(terms, em, rowv[:, 1, :])
            nc.vector.reduce_sum(s1, terms, axis=AX)
            rs1 = small_pool.tile([1, 1], fp32)
            nc.vector.reciprocal(rs1, s1)
            q0 = small_pool.tile([1, 1], fp32)
            nc.scalar.activation(
                out=q0, in_=bl, func=Act.Exp, bias=negb1, scale=1.0
            )
            q = small_pool.tile([1, 1], fp32)
            nc.vector.tensor_scalar(
                out=q, in0=q0, scalar1=sl, scalar2=rs1,
                op0=mybir.AluOpType.mult, op1=mybir.AluOpType.mult,
            )

            # fixup + output, sliced and split across engines/queues so the
            # out DMAs overlap the remaining fixup work; DVE is ~1.6x
            # faster per element than ACT, so it gets 4 of 6 slices
            fin_row = persist_pool.tile([1, S_PER], fp32)
            out2 = out.rearrange("(o s) -> o s", o=1)
            QN = S_PER // 8
            slices = [(0, 2), (2, 3), (3, 5), (5, 6), (6, 8)]
            for i, (a, b) in enumerate(slices):
                src = attn_row[:, a * QN : b * QN]
                dst = fin_row[:, a * QN : b * QN]
                if i in (1, 3):
                    nc.scalar.mul(dst, src, q)
                else:
                    nc.vector.tensor_scalar_mul(dst, src, q)
                deng = nc.sync if i % 2 == 0 else nc.scalar
                deng.dma_start(out=out2[:, a * QN : b * QN], in_=dst)

    nc.compile()
    return nc


def _get_module():
    if "nc" not in _CACHE:
        _CACHE["nc"] = _build_module()
    return _CACHE["nc"]


def _prep_inputs(hidden: np.ndarray, encoder_outputs: np.ndarray):
    hidden = np.asarray(hidden, dtype=np.float32)
    eo = np.asarray(encoder_outputs, dtype=np.float32)
    h16 = hidden.astype(np.float16)
    eo16 = eo.astype(np.float16)
    hcm = np.ascontiguousarray(h16.reshape(HC, P).T)  # [P, HC]
    in_maps = []
    for c in range(N_CORES):
        es = eo16[c * S_PER : (c + 1) * S_PER]  # [S_PER, H]
        blocks = [
            es[OFFS[i] : OFFS[i] + SIZES[i], :]
            .reshape(SIZES[i], HC, P)
            .transpose(2, 1, 0)
            .reshape(P, HC * SIZES[i])
            for i in range(NCH)
        ]
        ets = np.ascontiguousarray(np.concatenate(blocks, axis=1))  # [P, HC*S_PER]
        in_maps.append({"et": ets, "hc": hcm})
    return in_maps


def _run(hidden: np.ndarray, encoder_outputs: np.ndarray, trace: bool = False):
    from concourse.bass_utils import run_bass_kernel_spmd

    nc = _get_module()
    in_maps = _prep_inputs(hidden, encoder_outputs)
    res = run_bass_kernel_spmd(
        nc, in_maps, core_ids=list(range(N_CORES)), trace=trace
    )
    parts = [np.asarray(res.results[c]["out"]).reshape(-1) for c in range(N_CORES)]
    attn = np.concatenate(parts)
    return attn.reshape(1, 1, S_TOTAL).astype(np.float32), res


def kernel(hidden: np.ndarray, encoder_outputs: np.ndarray) -> np.ndarray:
    try:
        out, _ = _run(hidden, encoder_outputs, trace=False)
    except Exception:
        # one retry for transient device/runtime hiccups
        _CACHE.clear()
        out, _ = _run(hidden, encoder_outputs, trace=False)
    return out


# revision 17
# speedup vs baseline: 1.2613x; 1.1411x over previous
"""Trainium2 Bass kernel for attention-weight computation.

Computes attn = softmax(encoder_outputs @ hidden) over seq_len=65536,
returning shape (1, 1, 65536) float32.

Distribution: encoder_outputs [65536, 1024] is sharded by rows across 8
NeuronCores (8192 rows each).  The host casts each slice to fp16 (accuracy
checked: rel_l2 ~3e-5 vs fp32 reference, far under the 2e-2 gate — softmax
normalization cancels the dominant entry's quantization error) and
pre-tiles it so every DMA reads fully contiguous 8 KiB partition lines:
per chunk c of `size` seq columns, the host stores [128, HC*size] fp16
where partition p holds h-chunk-major data etile[p, j*size+s] =
E[base+s, j*128+p].  The core streams these chunks from HBM and computes
its 8192 scores on the TensorEngine (hidden chunk = 1-column stationary
operand, chunk tile = [128, <=512] moving operand, accumulating the 8
h-chunks into a [1, size] PSUM tile).

Softmax stats are computed incrementally during the stream (per-chunk max
+ sum-of-exp directly from PSUM on the otherwise-idle Vector/Act
engines).  One AllGather of the first 17 chunks' (m, s) pairs overlaps
the stream tail; a second tiny AllGather covers the last (small) chunk.
The tail combines the 144 gathered pairs into the global max g and sum S,
then rescales the stored exp(s - m_t) rows by exp(m_t - g)/S split across
the Vector and Act engines, and writes the result out.
"""

import numpy as np

S_TOTAL = 65536
H = 1024
N_CORES = 8
S_PER = S_TOTAL // N_CORES  # 8192 rows per core
P = 128                     # SBUF partitions
HC = H // P                 # 8 h-chunks

# chunk sizes: small first chunks for pipeline ramp, small last chunk so
# the final stats (which gate the tail AllGather) are ready quickly.
# 512-col subchunks are processed in pairs sharing the h-loop (two PSUM
# accumulation chains) so consecutive matmuls hit different banks and
# pipeline instead of paying the isolated-matmul drain each time.
SIZES = [128, 256] + [512] * 14 + [256, 256, 128]
assert sum(SIZES) == S_PER
OFFS = [sum(SIZES[:i]) for i in range(len(SIZES))]
NCH = len(SIZES)            # 19 chunks
# DMA groups: chunk indices loaded in one dma_start (pairs of 512s)
DMA_GROUPS = [[0], [1], [2, 3], [4, 5], [6, 7], [8, 9], [10, 11], [12, 13],
              [14, 15], [16, 17], [18]]
assert sorted(c for g in DMA_GROUPS for c in g) == list(range(NCH))

_CACHE: dict = {}


def _build_module(mm_dtype: str = "float16"):
    import concourse.bacc as bacc
    import concourse.mybir as mybir
    import concourse.tile as tile

    fp32 = mybir.dt.float32
    mmdt = getattr(mybir.dt, mm_dtype)
    AX = mybir.AxisListType.X
    ALL_CORES = [list(range(N_CORES))]
    Act = mybir.ActivationFunctionType

    nc = bacc.Bacc(
        "TRN2",
        target_bir_lowering=False,
        debug=False,
        enable_asserts=False,
        num_devices=N_CORES,
    )

    # et: pre-tiled slice [P, HC*S_PER]; hc: hidden as [P, HC] (chunk j in col j)
    et = nc.dram_tensor("et", [P, HC * S_PER], mmdt, kind="ExternalInput").ap()
    hc = nc.dram_tensor("hc", [P, HC], mmdt, kind="ExternalInput").ap()
    out = nc.dram_tensor("out", [S_PER], fp32, kind="ExternalOutput").ap()

    with tile.TileContext(nc) as tc:
        with (
            tc.tile_pool(name="stream", bufs=6) as stream_pool,
            tc.tile_pool(name="persist", bufs=1) as persist_pool,
            tc.tile_pool(name="small", bufs=1) as small_pool,
            tc.tile_pool(name="psum", bufs=6, space="PSUM") as psum_pool,
            tc.tile_pool(name="dram", bufs=1, space="DRAM") as dram_pool,
        ):
            hid = small_pool.tile([P, HC], mmdt)
            nc.sync.dma_start(out=hid, in_=hc)

            exps_row = persist_pool.tile([1, S_PER], fp32)   # exp(s - m_t)
            attn_row = persist_pool.tile([1, S_PER], fp32)
            pair_row = small_pool.tile([1, 2 * NCH], fp32)   # (m_t, sum_t)
            negm_row = small_pool.tile([1, NCH], fp32)

            cc_in = dram_pool.tile([2], fp32)
            cc_out = dram_pool.tile([N_CORES, 2], fp32)

            # ---- stream chunks: matmul + incremental softmax stats ----
            for gi, group in enumerate(DMA_GROUPS):
                g0, gsz = OFFS[group[0]], sum(SIZES[c] for c in group)
                etile = stream_pool.tile(
                    [P, HC * gsz], mmdt, tag="et", bufs=6, name=f"et{gi}"
                )
                eng = nc.sync if gi % 2 == 0 else nc.scalar
                eng.dma_start(
                    out=etile, in_=et[:, HC * g0 : HC * (g0 + gsz)]
                )
                # interleave the chunks' PSUM chains so consecutive
                # matmuls target different banks and pipeline
                pss = {
                    c: psum_pool.tile([1, SIZES[c]], fp32, tag="ps", bufs=6,
                                      name=f"ps{c}")
                    for c in group
                }
                for j in range(HC):
                    for c in group:
                        base = HC * (OFFS[c] - g0) + j * SIZES[c]
                        nc.tensor.matmul(
                            pss[c],
                            hid[:, j : j + 1],
                            etile[:, base : base + SIZES[c]],
                            start=(j == 0),
                            stop=(j == HC - 1),
                        )
                for c in group:
                    # stats straight from PSUM; ACT writes exp row + sum
                    nc.vector.reduce_max(pair_row[:, 2 * c : 2 * c + 1], pss[c], axis=AX)
                    nc.vector.tensor_scalar_mul(
                        negm_row[:, c : c + 1], pair_row[:, 2 * c : 2 * c + 1], -1.0
                    )
                    nc.scalar.activation(
                        out=exps_row[:, OFFS[c] : OFFS[c] + SIZES[c]],
                        in_=pss[c],
                        func=Act.Exp,
                        bias=negm_row[:, c : c + 1],
                        scale=1.0,
                        accum_out=pair_row[:, 2 * c + 1 : 2 * c + 2],
                    )

            # ---- local combine (b_l, S_l): needed both for the
            # speculative rescale and as the (tiny) AllGather payload ----
            pairv = pair_row.rearrange("o (k two) -> o two k", two=2)
            bl = small_pool.tile([1, 1], fp32)
            nc.vector.reduce_max(bl, pairv[:, 0, :], axis=AX)
            negbl = small_pool.tile([1, 1], fp32)
            nc.vector.tensor_scalar_mul(negbl, bl, -1.0)
            eml = small_pool.tile([1, NCH], fp32)
            nc.scalar.activation(
                out=eml, in_=pairv[:, 0, :], func=Act.Exp, bias=negbl, scale=1.0
            )
            termsl = small_pool.tile([1, NCH], fp32)
            sl = small_pool.tile([1, 1], fp32)
            nc.vector.tensor_mul(termsl, eml, pairv[:, 1, :])
            nc.vector.reduce_sum(sl, termsl, axis=AX)
            blsl = small_pool.tile([1, 2], fp32)
            nc.vector.tensor_copy(blsl[:, 0:1], bl)
            nc.vector.tensor_copy(blsl[:, 1:2], sl)
            nc.sync.dma_start(out=cc_in, in_=blsl)
            nc.gpsimd.collective_compute(
                "AllGather",
                mybir.AluOpType.bypass,
                replica_groups=ALL_CORES,
                ins=[cc_in.opt()],
                outs=[cc_out.opt()],
            )

            # ---- speculative softmax from LOCAL stats, fully overlapped
            # with the AllGather flight; exact global fixup afterwards.
            # attn_true = exps*exp(m_t - b)/S_b for ANY reference b, so
            # rescale with local (b_l, S_l) now and multiply by
            # q = exp(b_l - b)*S_l/S_b once global stats arrive. ----
            rsl = small_pool.tile([1, 1], fp32)
            nc.vector.reciprocal(rsl, sl)
            ftl = small_pool.tile([1, NCH], fp32)
            nc.vector.tensor_scalar_mul(ftl, eml, rsl)
            for c, size in enumerate(SIZES):
                src = exps_row[:, OFFS[c] : OFFS[c] + size]
                dst = attn_row[:, OFFS[c] : OFFS[c] + size]
                if c % 2 == 0:
                    nc.vector.tensor_scalar_mul(dst, src, ftl[:, c : c + 1])
                else:
                    nc.scalar.mul(dst, src, ftl[:, c : c + 1])

            # ---- global combine -> q, then one fixup multiply ----
            NP = N_CORES * 2                    # 16 gathered floats
            row = small_pool.tile([1, NP], fp32)
            nc.scalar.dma_start(out=row, in_=cc_out.rearrange("a b -> (a b)"))
            rowv = row.rearrange("o (k two) -> o two k", two=2)
            b1 = small_pool.tile([1, 1], fp32)
            nc.vector.reduce_max(b1, rowv[:, 0, :], axis=AX)
            negb1 = small_pool.tile([1, 1], fp32)
            nc.vector.tensor_scalar_mul(negb1, b1, -1.0)
            em = small_pool.tile([1, NP // 2], fp32)
            nc.scalar.activation(
                out=em, in_=rowv[:, 0, :], func=Act.Exp, bias=negb1, scale=1.0
            )
            terms = small_pool.tile([1, NP // 2], fp32)
            s1 = small_pool.tile([1, 1], fp32)
            nc.vector.tensor_mul(terms, em, rowv[:, 1, :])
            nc.vector.reduce_sum(s1, terms, axis=AX)
            rs1 = small_pool.tile([1, 1], fp32)
            nc.vector.reciprocal(rs1, s1)
            q0 = small_pool.tile([1, 1], fp32)
            nc.scalar.activation(
                out=q0, in_=bl, func=Act.Exp, bias=negb1, scale=1.0
            )
            q = small_pool.tile([1, 1], fp32)
            nc.vector.tensor_scalar(
                out=q, in0=q0, scalar1=sl, scalar2=rs1,
                op0=mybir.AluOpType.mult, op1=mybir.AluOpType.mult,
            )

            # fixup + output, sliced and split across engines/queues so the
            # out DMAs overlap the remaining fixup work; DVE is ~1.6x
            # faster per element than ACT, so it gets 4 of 6 slices
            fin_row = persist_pool.tile([1, S_PER], fp32)
            out2 = out.rearrange("(o s) -> o s", o=1)
            QN = S_PER // 8
            slices = [(0, 2), (2, 3), (3, 5), (5, 6), (6, 8)]
            for i, (a, b) in enumerate(slices):
                src = attn_row[:, a * QN : b * QN]
                dst = fin_row[:, a * QN : b * QN]
                if i in (1, 3):
                    nc.scalar.mul(dst, src, q)
                else:
                    nc.vector.tensor_scalar_mul(dst, src, q)
                deng = nc.sync if i % 2 == 0 else nc.scalar
                deng.dma_start(out=out2[:, a * QN : b * QN], in_=dst)

    nc.compile()
    return nc


def _get_module():
    if "nc" not in _CACHE:
        _CACHE["nc"] = _build_module()
    return _CACHE["nc"]


def _prep_inputs(hidden: np.ndarray, encoder_outputs: np.ndarray):
    hidden = np.asarray(hidden, dtype=np.float32)
    eo = np.asarray(encoder_outputs, dtype=np.float32)
    h16 = hidden.astype(np.float16)
    eo16 = eo.astype(np.float16)
    hcm = np.ascontiguousarray(h16.reshape(HC, P).T)  # [P, HC]
    in_maps = []
    for c in range(N_CORES):
        es = eo16[c * S_PER : (c + 1) * S_PER]  # [S_PER, H]
        blocks = [
            es[OFFS[i] : OFFS[i] + SIZES[i], :]
            .reshape(SIZES[i], HC, P)
            .transpose(2, 1, 0)
            .reshape(P, HC * SIZES[i])
            for i in range(NCH)
        ]
        ets = np.ascontiguousarray(np.concatenate(blocks, axis=1))  # [P, HC*S_PER]
        in_maps.append({"et": ets, "hc": hcm})
    return in_maps


def _run(hidden: np.ndarray, encoder_outputs: np.ndarray, trace: bool = False):
    from concourse.bass_utils import run_bass_kernel_spmd

    nc = _get_module()
    in_maps = _prep_inputs(hidden, encoder_outputs)
    res = run_bass_kernel_spmd(
        nc, in_maps, core_ids=list(range(N_CORES)), trace=trace
    )
    parts = [np.asarray(res.results[c]["out"]).reshape(-1) for c in range(N_CORES)]
    attn = np.concatenate(parts)
    return attn.reshape(1, 1, S_TOTAL).astype(np.float32), res


def kernel(hidden: np.ndarray, encoder_outputs: np.ndarray) -> np.ndarray:
    try:
        out, _ = _run(hidden, encoder_outputs, trace=False)
    except Exception:
        # one retry for transient device/runtime hiccups
        _CACHE.clear()
        out, _ = _run(hidden, encoder_outputs, trace=False)
    return out
